# revision 26
# baseline (speedup 1.0000x reference)
"""AdaConv2d fused kernel for 8 TRN2 NeuronCores (pure data parallel).

Per-sample pipeline (all fused on-chip):
  1. instance-norm stats (mean/var over HW)
  2. dynamic per-(b,c) depthwise 3x3 conv with reflect padding
  3. per-(b,c) scale+bias (folded algebraically into the depthwise taps:
     y = A*(sum_t w_t * x_t) + B with A = rstd*w_pt, B = bias - mu*A*sum(w))
  4. fixed 3x3 conv (256->256) with reflect padding, as 18 accumulated
     bf16 matmuls per PSUM block

Layout: channels on partitions (2 tiles of 128), pixels on the free axis.
Padded images are 66 rows x 66 cols stored flat with a 2-element leading
margin (so every depthwise tap and every matmul rhs is a fully CONTIGUOUS
1D slice).  flat(r, c) = 2 + 66*r + c.  Rows 0/65 and cols 0/65 are the
reflect pads.  A one-element-left-shifted copy (xpb2[i] = xpb[i+1]) keeps
all odd-offset depthwise taps 4-byte aligned for the DVE bf16 2x mode.
"""

import os
from contextlib import ExitStack

import numpy as np

B_GLOBAL = 32
N_CORES = 8
NB = B_GLOBAL // N_CORES  # batches per core
C = 256
H = W = 64
WPAD = W + 2        # 66 padded row length
HPAD = H + 2        # 66 padded rows
MARG = 2            # leading margin so tap windows stay in-bounds
FLAT = MARG + HPAD * WPAD + 2   # 4360 flat elements per padded image
NPIX = H * W        # 4096
CT = C // 128       # channel tiles
OT = C // 128       # out-channel tiles
EPS = 1e-5
BLK_ROWS = 8        # output rows per PSUM block (8*64=512 fp32, 3D-AP rhs)
USE_STT = False     # single fused MAC per tap; False -> mul+add pairs

ROW_BLOCKS = [(r0, BLK_ROWS) for r0 in range(0, H, BLK_ROWS)]

_CACHED = {}


def _build(nb=NB):
    import concourse.mybir as mybir
    import concourse.tile as tile
    from concourse import bacc

    f32 = mybir.dt.float32
    bf16 = mybir.dt.bfloat16
    AF = mybir.ActivationFunctionType
    ALU = mybir.AluOpType

    nc = bacc.Bacc(None, target_bir_lowering=False)

    x_ext = nc.declare_dram_parameter("x", [nb, C, H, W], f32, isOutput=False)
    wsp_ext = nc.declare_dram_parameter("wsp", [nb, CT, 128, 9], f32, isOutput=False)
    wpt_ext = nc.declare_dram_parameter("wpt", [nb, CT, 128], f32, isOutput=False)
    bis_ext = nc.declare_dram_parameter("bis", [nb, CT, 128], f32, isOutput=False)
    cw_ext = nc.declare_dram_parameter("cw", [CT, 128, 3, 3, OT, 128], bf16, isOutput=False)
    cb_ext = nc.declare_dram_parameter("cb", [OT, 128], f32, isOutput=False)
    out_ext = nc.declare_dram_parameter("out", [nb, C, H, W], f32, isOutput=True)

    with tile.TileContext(nc) as tc, ExitStack() as ctx:
        singles = ctx.enter_context(tc.tile_pool(name="singles", bufs=1))
        xin_pool = ctx.enter_context(tc.tile_pool(name="xin", bufs=2))
        xpb_pool = ctx.enter_context(tc.tile_pool(name="xpb", bufs=2))
        xpb2_pool = ctx.enter_context(tc.tile_pool(name="xpb2", bufs=2))
        yp_pool = ctx.enter_context(tc.tile_pool(name="yp", bufs=4))
        stage_pool = ctx.enter_context(tc.tile_pool(name="stage", bufs=2))
        small_pool = ctx.enter_context(tc.tile_pool(name="small", bufs=4))
        tmp_pool = ctx.enter_context(tc.tile_pool(name="tmp", bufs=2))
        psum_pool = ctx.enter_context(tc.tile_pool(name="psum", bufs=8, space="PSUM"))

        # ---- constants / fixed weights ----
        # (emitted lazily AFTER batch 0's x DMAs so the 1.2MB weight load
        # doesn't delay the first tile's data; weights are only needed at
        # ~50us when the first matmul fires)
        cw_sb = []

        def load_cw():
            for ct in range(CT):
                t = singles.tile([128, 3, 3, OT, 128], bf16, tag=f"cw{ct}")
                nc.sync.dma_start(out=t[:], in_=cw_ext[ct])
                cw_sb.append(t)

        cb_sb = singles.tile([128, OT], f32, tag="cb")
        for ot in range(OT):
            nc.sync.dma_start(out=cb_sb[:, ot : ot + 1], in_=cb_ext[ot, :, None])
        eps_sb = singles.tile([128, 1], f32, tag="eps")
        nc.vector.memset(eps_sb[:], EPS)
        ident = singles.tile([128, 128], bf16, tag="ident")
        from concourse.masks import make_identity
        make_identity(nc, ident[:])
        # touch the Sqrt activation table once so its ~1.3us load happens
        # before the first tile's stats need it
        warm = singles.tile([128, 1], f32, tag="warm")
        nc.scalar.activation(out=warm[:], in_=eps_sb[:], func=AF.Sqrt, bias=eps_sb[:])

        yp_tiles = {}

        def grid(flat_ap):
            """(128, FLAT) flat padded buffer -> (128, 66, 66) image view."""
            return flat_ap[:, MARG : MARG + HPAD * WPAD].rearrange(
                "p (r c) -> p r c", c=WPAD)

        def fill_borders(buf):
            g = grid(buf[:])
            nc.scalar.copy(out=g[:, 1 : 1 + H, 0:1], in_=g[:, 1 : 1 + H, 2:3])
            nc.scalar.copy(out=g[:, 1 : 1 + H, 65:66], in_=g[:, 1 : 1 + H, 63:64])
            nc.scalar.copy(out=g[:, 0], in_=g[:, 2])
            nc.scalar.copy(out=g[:, HPAD - 1], in_=g[:, HPAD - 3])

        def produce_yp(b, ct):
            """norm + depthwise pipeline for one (batch, channel-tile)."""
            xf = xin_pool.tile([128, H, W], f32, tag="xf")
            nc.sync.dma_start(out=xf[:, : H // 2], in_=x_ext[b, ct * 128 : (ct + 1) * 128, : H // 2])
            nc.sync.dma_start(out=xf[:, H // 2 :], in_=x_ext[b, ct * 128 : (ct + 1) * 128, H // 2 :])

            wsp = small_pool.tile([128, 9], f32, tag="wsp")
            nc.sync.dma_start(out=wsp[:], in_=wsp_ext[b, ct])
            wpt = small_pool.tile([128, 1], f32, tag="wpt")
            nc.sync.dma_start(out=wpt[:], in_=wpt_ext[b, ct, :, None])
            bis = small_pool.tile([128, 1], f32, tag="bis")
            nc.sync.dma_start(out=bis[:], in_=bis_ext[b, ct, :, None])

            xpb = xpb_pool.tile([128, FLAT], bf16, tag="xpb")
            xpb2 = xpb2_pool.tile([128, FLAT], bf16, tag="xpb2")

            # stats: sum(x) fused into the f32->bf16 convert; sum(x^2) via
            # ACT Square writing into xpb2's buffer (overwritten later).
            # Both run in row-halves so ACT starts as soon as the first
            # half-DMA lands.
            sumx = small_pool.tile([128, 2], f32, tag="sumx")
            sumsq = small_pool.tile([128, 2], f32, tag="sumsq")
            xff = xf[:].rearrange("p a b -> p (a b)")
            for hh in range(2):
                lo = hh * (NPIX // 2)
                hi = lo + NPIX // 2
                nc.scalar.activation(
                    out=xpb2[:, lo:hi], in_=xff[:, lo:hi],
                    func=AF.Square, accum_out=sumsq[:, hh : hh + 1],
                )
            # margins stay finite (reads run into them)
            nc.vector.memset(xpb[:, 0:MARG], 0.0)
            nc.vector.memset(xpb[:, FLAT - 2 : FLAT], 0.0)
            for hh in range(2):
                nc.scalar.activation(
                    out=grid(xpb[:])[:, 1 + hh * (H // 2) : 1 + (hh + 1) * (H // 2), 1 : 1 + W],
                    in_=xf[:, hh * (H // 2) : (hh + 1) * (H // 2)],
                    func=AF.Copy, accum_out=sumx[:, hh : hh + 1],
                )
            fill_borders(xpb)

            # batch 0 / tile 0's depthwise runs on the (still idle) PE via
            # diagonal matmuls -- it needs no aligned shifted copy.
            on_pe = (b == 0 and ct == 0)
            if not on_pe:
                # shifted copy (one element left) for 4B-aligned odd-offset
                # taps. NOTE: gpsimd bulk ops are poison here -- they hold
                # the shared DVE/GpSimd SBUF port for their whole duration
                # and stall every DVE tensor_tensor op, so this stays on ACT.
                nc.scalar.copy(out=xpb2[:, 0 : FLAT - 2], in_=xpb[:, 1 : FLAT - 1])
                nc.vector.memset(xpb2[:, FLAT - 2 : FLAT], 0.0)

            # ---- stats finalize (tiny per-partition ops) ----
            sx = small_pool.tile([128, 1], f32, tag="sx")
            nc.vector.reduce_sum(sx[:], sumx[:], axis=mybir.AxisListType.X)
            sq = small_pool.tile([128, 1], f32, tag="sq")
            nc.vector.reduce_sum(sq[:], sumsq[:], axis=mybir.AxisListType.X)
            mu = small_pool.tile([128, 1], f32, tag="mu")
            nc.vector.tensor_scalar_mul(mu[:], sx[:], 1.0 / NPIX)
            m2 = small_pool.tile([128, 1], f32, tag="m2")
            nc.vector.tensor_scalar_mul(m2[:], sq[:], 1.0 / NPIX)
            musq = small_pool.tile([128, 1], f32, tag="musq")
            nc.vector.tensor_mul(musq[:], mu[:], mu[:])
            var = small_pool.tile([128, 1], f32, tag="var")
            nc.vector.tensor_sub(var[:], m2[:], musq[:])
            std = small_pool.tile([128, 1], f32, tag="std")
            nc.scalar.activation(out=std[:], in_=var[:], func=AF.Sqrt, bias=eps_sb[:])
            rstd = small_pool.tile([128, 1], f32, tag="rstd")
            nc.vector.reciprocal(out=rstd[:], in_=std[:])
            a_sc = small_pool.tile([128, 1], f32, tag="a_sc")
            nc.vector.tensor_mul(a_sc[:], rstd[:], wpt[:])
            sw = small_pool.tile([128, 1], f32, tag="sw")
            nc.vector.reduce_sum(sw[:], wsp[:], axis=mybir.AxisListType.X)
            t1 = small_pool.tile([128, 1], f32, tag="t1")
            nc.vector.tensor_mul(t1[:], mu[:], a_sc[:])
            nc.vector.tensor_mul(t1[:], t1[:], sw[:])
            bconst = small_pool.tile([128, 1], f32, tag="bconst")
            nc.vector.tensor_sub(bconst[:], bis[:], t1[:])
            wsc = small_pool.tile([128, 9], f32, tag="wsc")
            nc.vector.tensor_scalar_mul(wsc[:], wsp[:], a_sc[:])

            # ---- depthwise: 9 taps on DVE over one flat 1D range ----
            # out positions [68, 4292) cover the whole interior; junk lands
            # on pad positions and is overwritten by fill_borders(yp).
            OUT0, OLEN = 68, 4224
            taps = [(dh, dw) for dh in (-1, 0, 1) for dw in (-1, 0, 1)]
            yp = yp_pool.tile([128, FLAT], bf16, tag="yp")
            yp_seg = yp[:, OUT0 : OUT0 + OLEN]

            def tap_src(dh, dw):
                t_off = WPAD * dh + dw
                if dw == 0:
                    s = OUT0 + t_off       # even
                    return xpb[:, s : s + OLEN]
                s = OUT0 - 1 + t_off       # even (t_off odd)
                return xpb2[:, s : s + OLEN]

            if on_pe:
                # 9 diagonal-lhsT matmuls per 512-pixel segment, accumulated
                # in PSUM; DVE drains psum + B into yp.  Also warms HAM
                # before the big conv starts.
                diagw = small_pool.tile([128, 9, 128], bf16, tag="diagw")
                for t in range(9):
                    nc.vector.tensor_scalar_mul(
                        diagw[:, t, :], ident[:], wsc[:, t : t + 1])
                s = OUT0
                while s < OUT0 + OLEN:
                    n = min(512, OUT0 + OLEN - s)
                    ps = psum_pool.tile([128, 512], f32, tag="ps",
                                        name=f"psdw_{b}_{ct}_{s}")
                    for t, (dh, dw) in enumerate(taps):
                        toff = WPAD * dh + dw
                        nc.tensor.matmul(
                            ps[:, :n], diagw[:, t, :], xpb[:, s + toff : s + toff + n],
                            start=(t == 0), stop=(t == 8),
                        )
                    nc.vector.tensor_scalar_add(yp[:, s : s + n], ps[:, :n], bconst[:])
                    s += n
                nc.vector.memset(yp[:, 0:MARG], 0.0)
                nc.vector.memset(yp[:, FLAT - 2 : FLAT], 0.0)
                fill_borders(yp)
                yp_tiles[(b, ct)] = yp
                return

            # center tap's multiply runs on ACT (it has slack); the other 8
            # taps' products come from DVE 4x-mode tensor_scalar muls, then
            # are combined with a pairwise ADD TREE (same op count as a
            # serial chain but 4x shorter dependency depth and ~2x better
            # bf16 rounding error).
            act_tap = (0, 0)
            t_act = (act_tap[0] + 1) * 3 + (act_tap[1] + 1)
            tmpa = tmp_pool.tile([128, OLEN], bf16, tag="dwtmpa")
            nc.scalar.mul(tmpa[:], tap_src(*act_tap), wsc[:, t_act : t_act + 1])

            # tap0 writes yp_seg = w0*x0 + B directly
            d0, w0 = taps[0]
            t0 = (d0 + 1) * 3 + (w0 + 1)
            nc.vector.tensor_scalar(
                yp_seg, tap_src(d0, w0), wsc[:, t0 : t0 + 1], bconst[:],
                op0=ALU.mult, op1=ALU.add,
            )
            rest = [t for t in taps[1:] if t != act_tap]  # 7 taps

            def mul_into(buf, tap):
                t = (tap[0] + 1) * 3 + (tap[1] + 1)
                nc.vector.tensor_scalar_mul(buf[:], tap_src(*tap), wsc[:, t : t + 1])

            ta = tmp_pool.tile([128, OLEN], bf16, tag="dwA", bufs=1, name=f"dwA_{b}_{ct}")
            tb = tmp_pool.tile([128, OLEN], bf16, tag="dwB", bufs=1, name=f"dwB_{b}_{ct}")
            tc_ = tmp_pool.tile([128, OLEN], bf16, tag="dwC", bufs=1, name=f"dwC_{b}_{ct}")
            mul_into(ta, rest[0])
            mul_into(tb, rest[1])
            nc.vector.tensor_add(ta[:], ta[:], tb[:])
            mul_into(tb, rest[2])
            mul_into(tc_, rest[3])
            nc.vector.tensor_add(tb[:], tb[:], tc_[:])
            nc.vector.tensor_add(ta[:], ta[:], tb[:])        # taps 1-4
            mul_into(tb, rest[4])
            mul_into(tc_, rest[5])
            nc.vector.tensor_add(tb[:], tb[:], tc_[:])
            mul_into(tc_, rest[6])
            nc.vector.tensor_add(tc_[:], tc_[:], tmpa[:])    # + ACT tap
            nc.vector.tensor_add(tb[:], tb[:], tc_[:])       # taps 5-7 + act
            nc.vector.tensor_add(yp_seg, yp_seg, ta[:])
            nc.vector.tensor_add(yp_seg, yp_seg, tb[:])

            nc.vector.memset(yp[:, 0:MARG], 0.0)
            nc.vector.memset(yp[:, FLAT - 2 : FLAT], 0.0)
            fill_borders(yp)
            yp_tiles[(b, ct)] = yp

        def big_conv(b):
            for ot in range(OT):
                stage = stage_pool.tile([128, H, W], f32, tag="stage")
                # groups of <=4 row-blocks so PSUM drain overlaps accumulate
                if b == 0:
                    # ct-outer: all 72 ct0 matmuls (8 banks x 9 taps) run
                    # before any ct1 matmul, so PE has ~17us of work while
                    # DVE finishes yp[ct1] -- removes the pipeline-ramp
                    # stalls on the first batch.
                    ps = {}
                    for r0, nr in ROW_BLOCKS:
                        ps[r0] = psum_pool.tile(
                            [128, BLK_ROWS * W], f32, tag="ps",
                            name=f"ps_{b}_{ot}_{r0}",
                        )
                    for ct in range(CT):
                        ypg = grid(yp_tiles[(b, ct)][:])
                        for dh in (-1, 0, 1):
                            for dw in (-1, 0, 1):
                                kh, kw = dh + 1, dw + 1
                                lhsT = cw_sb[ct][:, kh, kw, ot, :]
                                for r0, nr in ROW_BLOCKS:
                                    rhs = ypg[:, r0 + 1 + dh : r0 + 1 + dh + nr,
                                              1 + dw : 1 + dw + W]
                                    nc.tensor.matmul(
                                        ps[r0][:], lhsT, rhs,
                                        start=(ct == 0 and dh == -1 and dw == -1),
                                        stop=(ct == CT - 1 and dh == 1 and dw == 1),
                                    )
                    for r0, nr in ROW_BLOCKS:
                        src = ps[r0][:].rearrange("p (r c) -> p r c", c=W)
                        nc.scalar.activation(
                            out=stage[:, r0 : r0 + nr, :], in_=src,
                            func=AF.Identity, bias=cb_sb[:, ot : ot + 1],
                        )
                    nc.sync.dma_start(
                        out=out_ext[b, ot * 128 : (ot + 1) * 128], in_=stage[:],
                    )
                    continue
                # sequential per-bank accumulation: 18 back-to-back matmuls
                # into one bank, then drain; bench showed ~5% faster than
                # cycling 4 banks per weight.
                for r0, nr in ROW_BLOCKS:
                    ps = psum_pool.tile(
                        [128, BLK_ROWS * W], f32, tag="ps",
                        name=f"ps_{b}_{ot}_{r0}",
                    )
                    n_acc = CT * 9
                    i = 0
                    for ct in range(CT):
                        ypg = grid(yp_tiles[(b, ct)][:])
                        for dh in (-1, 0, 1):
                            for dw in (-1, 0, 1):
                                kh, kw = dh + 1, dw + 1
                                lhsT = cw_sb[ct][:, kh, kw, ot, :]
                                rhs = ypg[:, r0 + 1 + dh : r0 + 1 + dh + nr,
                                          1 + dw : 1 + dw + W]
                                nc.tensor.matmul(
                                    ps[:], lhsT, rhs,
                                    start=(i == 0), stop=(i == n_acc - 1),
                                )
                                i += 1
                    src = ps[:].rearrange("p (r c) -> p r c", c=W)
                    nc.scalar.activation(
                        out=stage[:, r0 : r0 + nr, :], in_=src,
                        func=AF.Identity, bias=cb_sb[:, ot : ot + 1],
                    )
                nc.sync.dma_start(
                    out=out_ext[b, ot * 128 : (ot + 1) * 128], in_=stage[:],
                )

        for b in range(nb):
            for ct in range(CT):
                produce_yp(b, ct)
                if b == 0 and ct == 0:
                    load_cw()  # after batch 0's x DMAs are queued
            big_conv(b)

    nc.compile()
    return nc


def _host_prep(x, w_spatial, w_pointwise, bias, conv_w, conv_b, nb=NB):
    import ml_dtypes

    ncores = x.shape[0] // nb
    cw = np.ascontiguousarray(
        conv_w.reshape(OT, 128, CT, 128, 3, 3).transpose(2, 3, 4, 5, 0, 1)
    ).astype(ml_dtypes.bfloat16)
    cb = np.ascontiguousarray(conv_b.reshape(OT, 128)).astype(np.float32)
    wsp = np.ascontiguousarray(w_spatial.reshape(-1, CT, 128, 9)).astype(np.float32)
    wpt = np.ascontiguousarray(w_pointwise.reshape(-1, CT, 128)).astype(np.float32)
    bis = np.ascontiguousarray(bias.reshape(-1, CT, 128)).astype(np.float32)
    x = np.ascontiguousarray(x).astype(np.float32)
    in_maps = []
    for i in range(ncores):
        sl = slice(i * nb, (i + 1) * nb)
        in_maps.append({
            "x": np.ascontiguousarray(x[sl]),
            "wsp": np.ascontiguousarray(wsp[sl]),
            "wpt": np.ascontiguousarray(wpt[sl]),
            "bis": np.ascontiguousarray(bis[sl]),
            "cw": cw,
            "cb": cb,
        })
    return in_maps


def _run(inputs, trace=False):
    from concourse.bass_utils import run_bass_kernel_spmd

    if "nc" not in _CACHED:
        _CACHED["nc"] = _build()
    nc = _CACHED["nc"]
    in_maps = _host_prep(**inputs)
    kw = {}
    if trace:
        import shutil
        tdir = "/tmp/kernel_trace_out"
        shutil.rmtree(tdir, ignore_errors=True)
        os.makedirs(tdir, exist_ok=True)
        kw["tmpdir"] = tdir
    res = run_bass_kernel_spmd(
        nc, in_maps, core_ids=list(range(N_CORES)), trace=trace, **kw
    )
    out = np.concatenate([res.results[i]["out"] for i in range(N_CORES)], axis=0)
    return out.astype(np.float32), res


def kernel(x, w_spatial, w_pointwise, bias, conv_w, conv_b):
    out, _ = _run(
        dict(x=np.asarray(x), w_spatial=np.asarray(w_spatial),
             w_pointwise=np.asarray(w_pointwise), bias=np.asarray(bias),
             conv_w=np.asarray(conv_w), conv_b=np.asarray(conv_b)),
        trace=bool(int(os.environ.get("KERNEL_TRACE", "0"))),
    )
    return out
